# revision 2
# baseline (speedup 1.0000x reference)
"""Handshaking kernel ('cat' type) for Trainium2, 8 NeuronCores.

Math: for each upper-triangular pair (i, j>=i):
    out[b, p(i,j), :] = tanh(W1 @ h_i + W2 @ h_j + bias),  W = [W1 | W2]

Decomposition: per-token projections A = seq @ W1^T + bias and C = seq @ W2^T.
This version keeps everything TRANSPOSED on device: hidden dim on partitions,
pair index on the free dim. For anchor i the pairs (i, j=i..255) are the
contiguous run [OFF[i], OFF[i]+256-i) of the flattened pair dim, so

    x^T[h, OFF[i]+m] = C^T[h, i+m] + A^T[h, i]

is ONE DVE tensor_scalar_add per run (in0 = contiguous C^T slice, scalar =
per-partition A^T column; fp16 hits the 4x DVE perf mode). ACT then applies
tanh over large staging spans and big contiguous DMAs write the transposed
output [384, 32896] fp16; the host only transposes back (no index gather).

Engine budget per core (cost model): DVE adds ~73us, ACT tanh ~86us
(bottleneck; tanh exists only on ACT), DMA out ~79us, PE projections ~7us,
all overlapped.

Sharding: 8 cores = 4 batches x 2 halves of the hidden dim (H=768 -> 384 per
core). All cores run the identical program (SPMD).
"""

import sys
import numpy as np

for _p in ("/opt/trn_rl_repo", "/root/.axon_site/_ro/trn_rl_repo"):
    if _p not in sys.path:
        sys.path.insert(0, _p)

B, L, H = 4, 256, 768
HH = H // 2            # per-core hidden slice
NCHUNK = HH // 128     # 3 partition chunks per core
NPAIR = L * (L + 1) // 2   # 32896
NSPAN = 4
SPAN = NPAIR // NSPAN  # 8224 staging/DMA span (16448B per-partition descs)
assert SPAN * NSPAN == NPAIR

# offset of pair (i, i) in the flattened pair dim; pair (i, j) -> OFF[i] + j - i
OFF = [i * L - (i * (i - 1)) // 2 for i in range(L)]

_CACHE = {}


def _span_segments():
    """Per span s: list of (i, c0, c1) meaning run i contributes columns
    [OFF[i]+c0, OFF[i]+c1) of the pair dim, clipped to the span."""
    segs = [[] for _ in range(NSPAN)]
    for i in range(L):
        lo, hi = OFF[i], OFF[i] + (L - i)
        for s in range(NSPAN):
            a, b = s * SPAN, (s + 1) * SPAN
            if hi <= a or lo >= b:
                continue
            segs[s].append((i, max(lo, a) - lo, min(hi, b) - lo))
    # coverage check
    tot = sum(c1 - c0 for sl in segs for (_, c0, c1) in sl)
    assert tot == NPAIR
    return segs


SEGS = _span_segments()


def _build_nc():
    import concourse.bass as bass  # noqa: F401
    import concourse.bacc as bacc
    import concourse.mybir as mybir
    import concourse.tile as tile

    f32 = mybir.dt.float32
    f32r = mybir.dt.float32r
    f16 = mybir.dt.float16
    Tanh = mybir.ActivationFunctionType.Tanh
    Ident = mybir.ActivationFunctionType.Identity

    nc = bacc.Bacc(None, target_bir_lowering=False, debug=False)

    # host-prepacked inputs: partition dim first, k-chunk-major free dim
    seqT = nc.dram_tensor("seqT", [128, 6 * L], f32r, kind="ExternalInput")
    w1t = nc.dram_tensor("w1t", [128, 6 * HH], f32r, kind="ExternalInput")
    w2t = nc.dram_tensor("w2t", [128, 6 * HH], f32r, kind="ExternalInput")
    biasc = nc.dram_tensor("biasc", [128, NCHUNK], f32, kind="ExternalInput")
    # transposed output: row h (within the core's 384-slice), col = pair idx
    out = nc.dram_tensor("out", [HH, NPAIR], f16, kind="ExternalOutput")

    with tile.TileContext(nc) as tc:
        with (
            tc.tile_pool(name="persist", bufs=1) as pers,
            tc.tile_pool(name="proj_ps", bufs=6, space="PSUM") as proj_ps,
            tc.tile_pool(name="pre", bufs=2) as prep,
            tc.tile_pool(name="post", bufs=2) as postp,
        ):
            seqT_sb = pers.tile([128, 6 * L], f32r, tag="seqT")
            w1t_sb = pers.tile([128, 6 * HH], f32r, tag="w1t")
            w2t_sb = pers.tile([128, 6 * HH], f32r, tag="w2t")
            bias_sb = pers.tile([128, NCHUNK], f32, tag="biasc")
            ct_sb = pers.tile([128, NCHUNK * L], f16, tag="Ct")
            at_sb = pers.tile([128, NCHUNK * L], f32, tag="At")

            nc.sync.dma_start(seqT_sb[:], seqT[:])
            nc.sync.dma_start(w1t_sb[:], w1t[:])
            nc.scalar.dma_start(w2t_sb[:], w2t[:])
            nc.scalar.dma_start(bias_sb[:], biasc[:])

            # ---- projections, transposed: X^T[h, i] per 128-row h-chunk ----
            # lhsT = W^T k-chunk [128k, 128h], rhs = seq^T k-chunk [128k, 256i]
            for c in range(NCHUNK):
                for wt_sb, dst, add_b in ((w2t_sb, ct_sb, False),
                                          (w1t_sb, at_sb, True)):
                    ps = proj_ps.tile([128, L], f32, tag="proj")
                    for k in range(6):
                        nc.tensor.matmul(
                            ps[:],
                            lhsT=wt_sb[:, k * HH + c * 128 : k * HH + (c + 1) * 128],
                            rhs=seqT_sb[:, k * L : (k + 1) * L],
                            start=(k == 0),
                            stop=(k == 5),
                        )
                    if add_b:
                        # A^T = psum + b[h] (per-partition bias column)
                        nc.scalar.activation(
                            dst[:, c * L : (c + 1) * L], ps[:], Ident,
                            bias=bias_sb[:, c : c + 1],
                        )
                    else:
                        nc.scalar.activation(
                            dst[:, c * L : (c + 1) * L], ps[:], Ident,
                        )

            # ---- main loop: per (chunk, span): DVE adds -> ACT tanh -> DMA --
            for c in range(NCHUNK):
                ct_c = ct_sb[:, c * L : (c + 1) * L]
                for s in range(NSPAN):
                    a = s * SPAN
                    pre = prep.tile([128, SPAN], f16, tag="pre")
                    for (i, c0, c1) in SEGS[s]:
                        nc.vector.tensor_scalar_add(
                            pre[:, OFF[i] + c0 - a : OFF[i] + c1 - a],
                            ct_c[:, i + c0 : i + c1],
                            at_sb[:, c * L + i : c * L + i + 1],
                        )
                    post = postp.tile([128, SPAN], f16, tag="post")
                    nc.scalar.activation(post[:], pre[:], Tanh)
                    nc.sync.dma_start(
                        out[c * 128 : (c + 1) * 128, a : a + SPAN], post[:]
                    )

    nc.compile()
    return nc


def _get_nc():
    if "nc" not in _CACHE:
        _CACHE["nc"] = _build_nc()
    return _CACHE["nc"]


def _pack_k(arr):
    """[768, n] -> [128, 6*n] with k-chunk-major free dim."""
    n = arr.shape[1]
    return np.ascontiguousarray(
        arr.reshape(6, 128, n).transpose(1, 0, 2).reshape(128, 6 * n)
    )


def make_in_maps(seq_hiddens, W, b):
    w1T = np.ascontiguousarray(W[:, :H].T)   # [k=768, h=768]
    w2T = np.ascontiguousarray(W[:, H:].T)
    in_maps = []
    for core in range(8):
        bb, hf = divmod(core, 2)
        hs = slice(hf * HH, (hf + 1) * HH)
        in_maps.append(
            {
                "seqT": _pack_k(np.ascontiguousarray(seq_hiddens[bb].T)),
                "w1t": _pack_k(w1T[:, hs]),
                "w2t": _pack_k(w2T[:, hs]),
                "biasc": np.ascontiguousarray(
                    b[hs].reshape(NCHUNK, 128).T
                ).astype(np.float32),
            }
        )
    return in_maps


def kernel(seq_hiddens, W, b):
    from concourse.bass_utils import run_bass_kernel_spmd

    seq_hiddens = np.asarray(seq_hiddens, dtype=np.float32)
    W = np.asarray(W, dtype=np.float32)
    b = np.asarray(b, dtype=np.float32)

    nc = _get_nc()
    in_maps = make_in_maps(seq_hiddens, W, b)
    res = run_bass_kernel_spmd(nc, in_maps, list(range(8)))
    full = np.empty((B, NPAIR, H), np.float32)
    for bb in range(B):
        for hf in range(2):
            buf = res.results[2 * bb + hf]["out"]  # [HH, NPAIR] fp16
            full[bb, :, hf * HH : (hf + 1) * HH] = buf.T.astype(np.float32)
    return full


if __name__ == "__main__":
    rng = np.random.RandomState(0)
    sh = rng.randn(B, L, H).astype(np.float32)
    Wv = (rng.randn(H, 2 * H) * 0.02).astype(np.float32)
    bv = np.zeros(H, np.float32)
    o = kernel(seq_hiddens=sh, W=Wv, b=bv)

    # host reference check
    ii, jj = np.triu_indices(L)
    A = sh @ Wv[:, :H].T + bv
    C = sh @ Wv[:, H:].T
    exp = np.tanh(A[:, ii, :] + C[:, jj, :])
    err = np.abs(o - exp).max()
    print("kernel output", o.shape, o.dtype, "abs err", err)
